# revision 1
# baseline (speedup 1.0000x reference)
"""Trainium2 Bass kernel for an 8-head MultiHeadAttention (b=8, s=1024, d=512).

Sharding: pure data-parallel over batch — each of the 8 NeuronCores runs the
full attention for one batch element. No collectives.

Per-core algorithm (matmul operands bf16, accumulate fp32):
  x^T, w^T built via PE transposes.
  Q^T[hd,s] = wq^T.T @ x^T   (scale 1/8 and bias folded into the PSUM->SBUF copy)
  K^T[hd,s] = wk^T.T @ x^T
  V[s,hd]   = x^T.T @ wv^T   (stored head-interleaved with a ones column per head)
  S^T[k,q]  = K_h^T.T @ Q_h^T                 (k positions on partitions)
  P^T       = exp(S^T) * (1-mask)^T           (== exp(masked scores): exp(-1e7)=0,
                                               and the reference's second masking
                                               pass is a no-op on exact zeros; max
                                               subtraction skipped - scores are O(1))
  O^T_h[65,q] = V_aug.T @ P^T  (row 64 = softmax denominator via the ones column)
  O^T_h[0:64] *= 1/denom  (outer-product broadcast via PE, multiply on DVE)
  out[q,d]  = O^T.T @ wo^T + bo
"""

import numpy as np

P = 128
S = 1024  # sequence length
D = 512  # d_model
H = 8  # heads
DK = 64  # head dim
CH = D // P  # 4 hd/dmodel chunks
ST = S // P  # 8 seq tiles
QC = S // 512  # 2 moving-dim chunks of 512
NCORES = 8

_CACHE = {}


def _build():
    import concourse.bacc as bacc
    import concourse.mybir as mybir
    import concourse.tile as tile
    from concourse.masks import make_identity

    f32 = mybir.dt.float32
    f32r = mybir.dt.float32r  # noqa: F841
    mmdt = mybir.dt.bfloat16
    AF = mybir.ActivationFunctionType
    OP = mybir.AluOpType

    def r(ap):
        return ap.bitcast(mmdt)

    nc = bacc.Bacc(None, target_bir_lowering=False, debug=False)

    x_t = nc.dram_tensor("x", [S, D], f32, kind="ExternalInput")
    mask_t = nc.dram_tensor("mask", [S, S], f32, kind="ExternalInput")
    wq_t = nc.dram_tensor("wq", [D, D], f32, kind="ExternalInput")
    wk_t = nc.dram_tensor("wk", [D, D], f32, kind="ExternalInput")
    wv_t = nc.dram_tensor("wv", [D, D], f32, kind="ExternalInput")
    wo_t = nc.dram_tensor("wo", [D, D], f32, kind="ExternalInput")
    bq_t = nc.dram_tensor("bq", [D], f32, kind="ExternalInput")
    bk_t = nc.dram_tensor("bk", [D], f32, kind="ExternalInput")
    bv_t = nc.dram_tensor("bv", [D], f32, kind="ExternalInput")
    bo_t = nc.dram_tensor("bo", [D], f32, kind="ExternalInput")
    out_t = nc.dram_tensor("out", [S, D], f32, kind="ExternalOutput")

    with tile.TileContext(nc) as tc:
        with (
            tc.tile_pool(name="persist", bufs=1) as pp,
            tc.tile_pool(name="fin", bufs=3) as fpool,
            tc.tile_pool(name="rcp", bufs=2) as rpool,
            tc.tile_pool(name="bcs", bufs=2) as bcsp,
            tc.tile_pool(name="pbig", bufs=2, space="PSUM") as pbig,
            tc.tile_pool(name="psmall", bufs=2, space="PSUM") as psmall,
            tc.tile_pool(name="pbc", bufs=2, space="PSUM") as pbc,
        ):
            # LIFO pool lifetimes (stack alloc):
            #   mid (om^T): t0 .. end
            #   maskp (mask_sb): t0 .. after om^T built
            #   stage (xT, wT-qkv, load chunks): t0 .. after projections
            #   late (oT) + ptp (P^T): after maskp .. end
            mid_cm = tc.tile_pool(name="mid", bufs=1)
            mid = mid_cm.__enter__()
            maskp_cm = tc.tile_pool(name="maskp", bufs=1)
            maskp = maskp_cm.__enter__()
            stage_cm = tc.tile_pool(name="stage", bufs=1)
            stage = stage_cm.__enter__()
            # ---- constants & small inputs ----
            ident = pp.tile([P, P], f32, name="id", tag="id")
            make_identity(nc, ident[:])
            ones_f32 = pp.tile([P, P], f32, name="ones_f32", tag="ones_f32")
            nc.vector.memset(ones_f32[:], 1.0)

            ones_sb = pp.tile([1, P], mmdt, name="ones", tag="ones")
            nc.vector.tensor_copy(ones_sb[:], ones_f32[0:1, :])

            bq_sb = pp.tile([P, CH], f32, name="bq", tag="bq")
            bk_sb = pp.tile([P, CH], f32, name="bk", tag="bk")
            nc.sync.dma_start(out=bq_sb[:], in_=bq_t[:].rearrange("(c p) -> p c", p=P))
            nc.sync.dma_start(out=bk_sb[:], in_=bk_t[:].rearrange("(c p) -> p c", p=P))
            qbias_sb = pp.tile([P, CH], f32, name="qbias", tag="qbias")
            nc.vector.tensor_scalar_mul(qbias_sb[:], bq_sb[:], 0.125)

            bv_bc = pp.tile([P, D], f32, name="bvbc", tag="bvbc")
            bo_bc = pp.tile([P, D], f32, name="bobc", tag="bobc")
            nc.gpsimd.dma_start(out=bv_bc[:], in_=bv_t[None, :].to_broadcast([P, D]))
            nc.gpsimd.dma_start(out=bo_bc[:], in_=bo_t[None, :].to_broadcast([P, D]))

            # ---- mask load (overlaps the compute below) ----
            mask_sb = maskp.tile([P, ST, S], f32, name="mask", tag="mask")
            nc.sync.dma_start(
                out=mask_sb[:], in_=mask_t[:].rearrange("(i p) k -> p i k", p=P)
            )

            # ---- load x / w in column-chunks, transpose on the fly ----
            xT = stage.tile([P, CH, S], mmdt, name="xT", tag="xT")
            for c in range(CH):
                xc = stage.tile([P, ST, P], f32, name="xc", tag="xc", bufs=2)
                nc.sync.dma_start(
                    out=xc[:],
                    in_=x_t[:, c * P : (c + 1) * P].rearrange("(i p) d -> p i d", p=P),
                )
                ps = pbig.tile([P, S], f32, name="sbig", tag="sbig")
                for i in range(ST):
                    nc.tensor.transpose(
                        ps[:, i * P : (i + 1) * P], xc[:, i, :], ident[:]
                    )
                nc.scalar.copy(xT[:, c, :], ps[:])

            wT = {}
            for name, t in (("wq", wq_t), ("wk", wk_t), ("wv", wv_t), ("wo", wo_t)):
                wT[name] = (stage if name != "wo" else pp).tile(
                    [P, CH, D], mmdt, name="T", tag="T" + name
                )
                for c in range(CH):
                    wc = stage.tile([P, CH, P], f32, name="wc", tag="wc", bufs=2)
                    nc.sync.dma_start(
                        out=wc[:],
                        in_=t[:, c * P : (c + 1) * P].rearrange("(r p) d -> p r d", p=P),
                    )
                    ps = pbig.tile([P, D], f32, name="sbig", tag="sbig")
                    for rr in range(CH):
                        nc.tensor.transpose(
                            ps[:, rr * P : (rr + 1) * P], wc[:, rr, :], ident[:]
                        )
                    nc.scalar.copy(wT[name][:, c, :], ps[:])

            # ---- projections Q^T, K^T ----
            qT = pp.tile([P, CH, S], mmdt, name="qT", tag="qT")
            kT = pp.tile([P, CH, S], mmdt, name="kT", tag="kT")
            for dst, wname, bias, scale in (
                (qT, "wq", qbias_sb, 0.125),
                (kT, "wk", bk_sb, 1.0),
            ):
                for c in range(CH):
                    ps = pbig.tile([P, S], f32, name="sbig", tag="sbig")
                    for j in range(QC):
                        for rr in range(CH):
                            nc.tensor.matmul(
                                ps[:, j * 512 : (j + 1) * 512],
                                wT[wname][:, rr, c * P : (c + 1) * P],
                                xT[:, rr, j * 512 : (j + 1) * 512],
                                start=(rr == 0),
                                stop=(rr == CH - 1),
                            )
                    nc.scalar.activation(
                        dst[:, c, :], ps[:], AF.Identity, bias=bias[:, c : c + 1], scale=scale
                    )

            # ---- projection V (head-interleaved, ones column per head) ----
            v_sb = pp.tile([P, ST, H * 65], mmdt, name="v", tag="v")
            nc.vector.tensor_copy(
                v_sb[:].rearrange("p i (h e) -> p i h e", e=65)[:, :, :, 64],
                ones_f32[:, 0 : ST * H].rearrange("p (i h) -> p i h", h=H),
            )
            for i in range(ST):
                ps = psmall.tile([P, 512], f32, name="sm", tag="sm")
                for rr in range(CH):
                    nc.tensor.matmul(
                        ps[:],
                        xT[:, rr, i * P : (i + 1) * P],
                        wT["wv"][:, rr, :],
                        start=(rr == 0),
                        stop=(rr == CH - 1),
                    )
                nc.vector.tensor_add(
                    v_sb[:, i, :].rearrange("p (h e) -> p h e", e=65)[:, :, 0:64],
                    ps[:].rearrange("p (h e) -> p h e", e=64),
                    bv_bc[:].rearrange("p (h e) -> p h e", e=64),
                )

            stage_cm.__exit__(None, None, None)

            # ---- om^T = (1 - mask)^T ----
            omT = mid.tile([P, ST, S], mmdt, name="omT", tag="omT")
            for kc in range(ST):
                ps = pbig.tile([P, S], f32, name="sbig", tag="sbig")
                for qi in range(ST):
                    nc.tensor.transpose(
                        ps[:, qi * P : (qi + 1) * P],
                        mask_sb[:, qi, kc * P : (kc + 1) * P],
                        ident[:],
                    )
                nc.vector.tensor_scalar(
                    omT[:, kc, :], ps[:], -1.0, 1.0, op0=OP.mult, op1=OP.add
                )

            maskp_cm.__exit__(None, None, None)
            late_cm = tc.tile_pool(name="late", bufs=1)
            late = late_cm.__enter__()
            ptp_cm = tc.tile_pool(name="ptp", bufs=2)
            ptp = ptp_cm.__enter__()

            # ---- attention heads ----
            oT = late.tile([P, CH, S], mmdt, name="oT", tag="oT")
            pts = {}

            def emit_sp(h):
                c, off = h // 2, 64 * (h % 2)
                qh = qT[off : off + 64, c, :]
                kh = kT[off : off + 64, c, :]
                pt = ptp.tile([P, ST, S], mmdt, name="pt", tag="pt")
                pts[h] = pt
                for kc in range(ST):
                    ps = pbig.tile([P, S], f32, name="sbig", tag="sbig")
                    for j in range(QC):
                        nc.tensor.matmul(
                            ps[:, j * 512 : (j + 1) * 512],
                            kh[:, kc * P : (kc + 1) * P],
                            qh[:, j * 512 : (j + 1) * 512],
                            start=True,
                            stop=True,
                        )
                    nc.scalar.activation(pt[:, kc, :], ps[:], AF.Exp)
                    eng = nc.gpsimd if kc == 5 else nc.vector
                    eng.tensor_mul(pt[:, kc, :], pt[:, kc, :], omT[:, kc, :])

            def emit_pv(h):
                c, off = h // 2, 64 * (h % 2)
                pt = pts.pop(h)
                for j in range(QC):
                    pv = psmall.tile([P, 512], f32, name="sm", tag="sm")
                    for kc in range(ST):
                        nc.tensor.matmul(
                            pv[0:65, :],
                            v_sb[:, kc, h * 65 : (h + 1) * 65],
                            pt[:, kc, j * 512 : (j + 1) * 512],
                            start=(kc == 0),
                            stop=(kc == ST - 1),
                        )
                    rc = rpool.tile([1, 512], mmdt, name="rc", tag="rc")
                    with nc.allow_low_precision(reason="bf16 recip feeds bf16 matmul"):
                        nc.vector.reciprocal(rc[:], pv[64:65, :])
                    bc = pbc.tile([P, 512], f32, name="bc", tag="bc")
                    nc.tensor.matmul(
                        bc[0:64, :], ones_sb[:, 0:64], rc[:], start=True, stop=True
                    )
                    bcs = bcsp.tile([64, 512], f32, name="bcsb", tag="bcsb")
                    nc.vector.tensor_copy(bcs[:], bc[0:64, :])
                    nc.vector.tensor_mul(
                        oT[off : off + 64, c, j * 512 : (j + 1) * 512],
                        pv[0:64, :],
                        bcs[:],
                    )

            for h in range(H):
                emit_sp(h)
                if h > 0:
                    emit_pv(h - 1)
            emit_pv(H - 1)

            # ---- output projection ----
            for qt in range(ST):
                pf = psmall.tile([P, 512], f32, name="sm", tag="sm")
                for c in range(CH):
                    nc.tensor.matmul(
                        pf[:],
                        oT[:, c, qt * P : (qt + 1) * P],
                        wT["wo"][:, c, :],
                        start=(c == 0),
                        stop=(c == CH - 1),
                    )
                ft = fpool.tile([P, 512], f32, name="fin", tag="fin")
                nc.vector.tensor_add(ft[:], pf[:], bo_bc[:])
                nc.sync.dma_start(out=out_t[qt * P : (qt + 1) * P, :], in_=ft[:])

            ptp_cm.__exit__(None, None, None)
            late_cm.__exit__(None, None, None)
            mid_cm.__exit__(None, None, None)
            # (maskp_cm was exited before `late` opened; mid spans to the end)

    nc.compile()
    return nc


def _get_nc():
    if "nc" not in _CACHE:
        _CACHE["nc"] = _build()
    return _CACHE["nc"]


def run(inputs, trace=False, **kw):
    from concourse.bass_utils import run_bass_kernel_spmd

    nc = _get_nc()
    f = np.float32
    in_maps = [
        {
            "x": np.ascontiguousarray(inputs["inputs"][i], dtype=f),
            "mask": np.ascontiguousarray(inputs["mask"][i], dtype=f),
            "wq": np.ascontiguousarray(inputs["wq"], dtype=f),
            "wk": np.ascontiguousarray(inputs["wk"], dtype=f),
            "wv": np.ascontiguousarray(inputs["wv"], dtype=f),
            "wo": np.ascontiguousarray(inputs["wo"], dtype=f),
            "bq": np.ascontiguousarray(inputs["bq"], dtype=f),
            "bk": np.ascontiguousarray(inputs["bk"], dtype=f),
            "bv": np.ascontiguousarray(inputs["bv"], dtype=f),
            "bo": np.ascontiguousarray(inputs["bo"], dtype=f),
        }
        for i in range(NCORES)
    ]
    res = run_bass_kernel_spmd(nc, in_maps, list(range(NCORES)), trace=trace, **kw)
    out = np.stack(
        [np.asarray(res.results[i]["out"], dtype=np.float32) for i in range(NCORES)],
        axis=0,
    )
    return out, res


def kernel(**inputs):
    out, _ = run(inputs)
    return out

